# revision 30
# baseline (speedup 1.0000x reference)
"""Trainium2 Bass kernel for dense_cnn problem.

Math (per batch element n, C=128 channels, H=W=56, G=8):
  t1 = conv_h(x, w1)          5-tap conv over H with full channel mixing
  t3 = dwconv_h(t1, w3)       3-tap depthwise conv over H
  t4[g] = sum_{c,k} x[c, h, w+2k-2] * w4[c,k,g]   (3 width taps, dil 2)
  out[c] = t3[c] * t4[c % 8]

Device strategy (data-parallel, 4 batch elems per core across 8 cores):
  - PE does ONLY the dense work: t1 as a 5-tap conv (clipped shifted
    matmuls) and t4 broadcast to 128 channels (3 taps) -> 8 column
    passes per chunk instead of the 10 the folded-7-tap version needs.
  - The 3-tap depthwise conv runs on the otherwise-idle vector engines.
    ScalarE makes two per-partition-scaled copies of psA:
      t1s = w3[c,1] * t1      v = w3[c,0] * t1
    then (TensorScalarPtr is not codegen-supported on GpSimd, plain
    tensor_tensor is):
      acc = v[h-1] + t1s[h]                            (GpSimd ADD)
      t3  = (w3[c,2]/w3[c,1]) * t1s[h+1] + acc         (DVE STT)
    t1s/v have zero pad rows, so no border special cases.
    t3 for chunk c needs t1s row h0+8 from chunk c+1's copy, so the
    dw/multiply/store pipeline runs one chunk behind the PE.
  - Final multiply on DVE reads psB (t4) straight from PSUM and writes
    fp16; output DMA'd as fp16 (half the bytes) and widened on host.
  - Matmuls in bf16 (fp32 matmul lowers to a LOW_HIGH pair at <half
    throughput); accumulation stays fp32 in PSUM.
  - Head: input DMAs issued finest-first (x batch 0 in 4 row-slices) so
    the first chunk's data lands ASAP; 7 dummy warm-up matmuls trip the
    PE_HAM clock gate (1.2 -> 2.4 GHz) while the DMAs stream.
  - Tail: the last batch elem stores per-chunk (not per-pair) to cut
    the post-last-matmul drain.
"""

import sys

sys.path.insert(0, "/opt/trn_rl_repo")

import ml_dtypes
import numpy as np

import concourse.bacc as bacc
import concourse.bass as bass
import concourse.mybir as mybir
import concourse.tile as tile
from concourse import bass_utils

N, C, H, W, G = 32, 128, 56, 56, 8
NCORES = 8
NPC = N // NCORES  # batch elems per core
CH = 8             # H rows per chunk
NCHUNK = H // CH

F32 = mybir.dt.float32
F16 = mybir.dt.float16
BF16 = mybir.dt.bfloat16

TRACE = False
TRACE_DIR = None
LAST_EXEC_NS = None
LAST_RESULTS = None

_COMPILED = None


def _enable_trace_hook():
    """The agent image's ``antenv`` lacks ``axon_hooks``, so the boot-time
    NTFF hook registration silently degraded. Recreate the module and
    register the same ctypes-based hook; also skip the bucket upload."""
    import sys as _sys
    import types

    if "antenv.axon_hooks" not in _sys.modules:
        mod = types.ModuleType("antenv.axon_hooks")
        mod._hook = None

        def set_axon_ntff_profile_hook(h):
            mod._hook = h

        def get_axon_ntff_profile_hook():
            return mod._hook

        mod.set_axon_ntff_profile_hook = set_axon_ntff_profile_hook
        mod.get_axon_ntff_profile_hook = get_axon_ntff_profile_hook
        _sys.modules["antenv.axon_hooks"] = mod
        import antenv

        antenv.axon_hooks = mod

    from antenv.axon_hooks import get_axon_ntff_profile_hook as _get

    if _get() is None:
        from trn_agent_boot.trn_boot import _ntff_profile_via_ctypes

        hook = _ntff_profile_via_ctypes("/opt/axon/libaxon_pjrt.so")
        if hook is not None:
            _sys.modules["antenv.axon_hooks"].set_axon_ntff_profile_hook(hook)

    bass_utils.upload_artifacts = lambda tmpdir: f"local:{tmpdir}"


def _t1_matmuls(c, pa, xc, wc_t):
    """5-tap H-conv for chunk c with row clipping at the H borders.
    Output row o of the chunk reads x row 8c+o+e-2 for tap e."""
    h0 = c * CH
    mms = []
    # e=2 covers the full chunk for every c -> emitted first (start=True)
    for e in (2, 0, 1, 3, 4):
        o_lo = max(0, 2 - e - h0)
        o_hi = min(CH, H + 2 - e - h0)
        if o_lo >= o_hi:
            continue
        r0 = h0 + o_lo + e - 2
        r1 = h0 + o_hi + e - 2
        mms.append((wc_t[:, e, :], xc[:, r0:r1, :], pa[:, o_lo:o_hi, :]))
    return mms


def _t4_matmuls(c, pb, xc, w4_t):
    """t4 chunk: 3 width taps at offsets -2/0/+2, col-clipped at borders."""
    h0 = c * CH
    rows = xc[:, h0 : h0 + CH, :]
    return [
        (w4_t[:, 1, :], rows, pb[:]),                               # delta = 0
        (w4_t[:, 0, :], xc[:, h0 : h0 + CH, 0 : W - 2], pb[:, :, 2:W]),   # -2
        (w4_t[:, 2, :], xc[:, h0 : h0 + CH, 2:W], pb[:, :, 0 : W - 2]),   # +2
    ]


def _build():
    nc = bacc.Bacc(
        "TRN2",
        target_bir_lowering=False,
        debug=False,
        enable_asserts=False,
        num_devices=NCORES,
    )

    x_d = nc.dram_tensor("x_s", (NPC, C, H, W), BF16, kind="ExternalInput").ap()
    wc_d = nc.dram_tensor("wc5", (C, 5, C), BF16, kind="ExternalInput").ap()
    w4_d = nc.dram_tensor("w4b", (C, 3, C), BF16, kind="ExternalInput").ap()
    sc_d = nc.dram_tensor("scal", (C, 3), F32, kind="ExternalInput").ap()
    out_d = nc.dram_tensor("out", (NPC, C, H, W), F16, kind="ExternalOutput").ap()

    mult = mybir.AluOpType.mult
    add = mybir.AluOpType.add
    COPY = mybir.ActivationFunctionType.Copy

    with tile.TileContext(nc) as tc:
        with (
            tc.tile_pool(name="wpool", bufs=1) as wpool,
            tc.tile_pool(name="xpool", bufs=1) as xpool,
            tc.tile_pool(name="t1pool", bufs=2) as t1pool,
            tc.tile_pool(name="accpool", bufs=3) as accpool,
            tc.tile_pool(name="t3pool", bufs=3) as t3pool,
            tc.tile_pool(name="opool", bufs=3) as opool,
            tc.tile_pool(name="psA", bufs=3, space="PSUM") as papool,
            tc.tile_pool(name="psB", bufs=5, space="PSUM") as pbpool,
        ):
            # Dummy matmuls while the first DMAs stream in: PE_HAM ungates
            # the 2.4 GHz clock only after ~3us of sustained activity.
            # Results land in a PSUM bank that is never read.
            dmy = wpool.tile([C, 512], BF16)
            nc.vector.memset(dmy[:], 0.0)
            dps = papool.tile([C, CH, W], F32, name="pa")
            for _ in range(7):
                nc.tensor.matmul(
                    dps[:], lhsT=dmy[:, 0:C], rhs=dmy[:, 0 : CH * W],
                    start=True, stop=True,
                )

            wc_t = wpool.tile([C, 5, C], BF16)
            w4_t = wpool.tile([C, 3, C], BF16)
            sc_t = wpool.tile([C, 3], F32)

            xcs = []
            for n in range(NPC):
                xc = xpool.tile([C, H, W], BF16, name=f"xc{n}")
                xcs.append(xc)

            # DMA order: weights first (needed by the very first LDWEIGHTS),
            # then batch 0 in fine row slices so chunk 0 can start ASAP.
            nc.sync.dma_start(wc_t[:], wc_d[:])
            nc.sync.dma_start(xcs[0][:, 0:14, :], x_d[0, :, 0:14, :])
            nc.sync.dma_start(sc_t[:], sc_d[:])
            nc.sync.dma_start(w4_t[:], w4_d[:])
            nc.sync.dma_start(xcs[0][:, 14:28, :], x_d[0, :, 14:28, :])
            nc.sync.dma_start(xcs[0][:, 28:42, :], x_d[0, :, 28:42, :])
            nc.sync.dma_start(xcs[0][:, 42:56, :], x_d[0, :, 42:56, :])
            for n in range(1, NPC):
                nc.sync.dma_start(xcs[n][:, 0:28, :], x_d[n, :, 0:28, :])
                nc.sync.dma_start(xcs[n][:, 28:56, :], x_d[n, :, 28:56, :])

            w31 = sc_t[:, 0:1]
            w30 = sc_t[:, 1:2]
            r2 = sc_t[:, 2:3]

            for n in range(NPC):
                xc = xcs[n]
                last_n = n == NPC - 1

                # t1s rows: 0 = zero pad (h=-1), 1..56 = h, 57 = zero pad
                t1s = t1pool.tile([C, H + 2, W], F32, name="t1s")
                nc.gpsimd.memset(t1s[:, 0:1, :], 0.0)
                nc.gpsimd.memset(t1s[:, H + 1 : H + 2, :], 0.0)
                # v rows: 0 = zero pad (h=-1), 1..56 = h
                v = t1pool.tile([C, H + 1, W], F32, name="v")
                nc.gpsimd.memset(v[:, 0:1, :], 0.0)

                accs = [None] * NCHUNK
                pbs = [None] * NCHUNK
                ots = [None] * NCHUNK

                def emit_front(c):
                    """PE matmuls + Act copy + GpSimd first dw op for chunk c."""
                    h0 = c * CH
                    pa = papool.tile([C, CH, W], F32, name="pa")
                    mms = _t1_matmuls(c, pa, xc, wc_t)
                    for i, (lhsT, rhs, outap) in enumerate(mms):
                        nc.tensor.matmul(
                            outap, lhsT=lhsT, rhs=rhs,
                            start=(i == 0), stop=(i == len(mms) - 1),
                        )
                    pb = pbpool.tile([C, CH, W], F32, name="pb")
                    for i, (lhsT, rhs, outap) in enumerate(_t4_matmuls(c, pb, xc, w4_t)):
                        nc.tensor.matmul(
                            outap, lhsT=lhsT, rhs=rhs,
                            start=(i == 0), stop=(i == 2),
                        )
                    pbs[c] = pb
                    # t1s[1+h0 : 1+h0+CH] = w3_1 * t1   (per-partition scale)
                    nc.scalar.activation(
                        t1s[:, 1 + h0 : 1 + h0 + CH, :], pa[:], COPY, scale=w31
                    )
                    # v[1+h0 : 1+h0+CH] = w3_0 * t1
                    nc.scalar.activation(
                        v[:, 1 + h0 : 1 + h0 + CH, :], pa[:], COPY, scale=w30
                    )
                    # acc = w3_0*t1[h-1] + w3_1*t1[h]
                    acc = accpool.tile([C, CH, W], F32, name="acc")
                    nc.gpsimd.tensor_add(
                        acc[:],
                        v[:, h0 : h0 + CH, :],
                        t1s[:, 1 + h0 : 1 + h0 + CH, :],
                    )
                    accs[c] = acc

                def emit_back(c):
                    """DVE second dw op + final multiply + output DMA for chunk c.
                    Requires chunk c+1's Act copy already emitted (reads t1s
                    row h0+8), except for the last chunk (zero pad row)."""
                    h0 = c * CH
                    t3 = t3pool.tile([C, CH, W], F32, name="t3")
                    nc.vector.scalar_tensor_tensor(
                        t3[:],
                        t1s[:, 2 + h0 : 2 + h0 + CH, :],
                        r2,
                        accs[c][:],
                        op0=mult, op1=add,
                    )
                    if last_n:
                        ot = opool.tile([C, CH, W], F16, name="ot")
                        nc.vector.tensor_mul(ot[:], t3[:], pbs[c][:])
                        nc.sync.dma_start(out_d[n, :, h0 : h0 + CH, :], ot[:])
                        ots[c] = ot
                    else:
                        if c % 2 == 0:
                            ots[c] = opool.tile([C, 2 * CH, W], F16, name="otp")
                        else:
                            ots[c] = ots[c - 1]
                        sl = ots[c][:, (c % 2) * CH : (c % 2 + 1) * CH, :]
                        nc.vector.tensor_mul(sl, t3[:], pbs[c][:])
                        if c % 2 == 1 or c == NCHUNK - 1:
                            p0 = (c // 2) * 2 * CH
                            rows = (c % 2 + 1) * CH
                            nc.sync.dma_start(
                                out_d[n, :, p0 : p0 + rows, :],
                                ots[c][:, 0:rows, :],
                            )

                for c in range(NCHUNK):
                    emit_front(c)
                    if c >= 1:
                        emit_back(c - 1)
                emit_back(NCHUNK - 1)

    nc.compile()
    return nc


def _get_compiled():
    global _COMPILED
    if _COMPILED is None:
        _COMPILED = _build()
    return _COMPILED


def _prep_weights(w1, w3, w4):
    bf = ml_dtypes.bfloat16
    w1c = np.asarray(w1, dtype=np.float32)[:, :, :, 0]  # (co, ci, 5)
    wc5 = np.ascontiguousarray(np.transpose(w1c, (1, 2, 0))).astype(bf)  # (ci,e,co)
    w4c = np.asarray(w4, dtype=np.float32)[:, :, 0, :]  # (ci, k, g)
    w4b = np.ascontiguousarray(np.tile(w4c, (1, 1, C // G))).astype(bf)
    w3c = np.asarray(w3, dtype=np.float32)[:, 0, :, 0]  # (co, 3)
    w31 = w3c[:, 1].copy()
    w31[np.abs(w31) < 1e-12] = 1e-12
    scal = np.stack([w31, w3c[:, 0], w3c[:, 2] / w31], axis=1)
    return wc5, w4b, np.ascontiguousarray(scal, dtype=np.float32)


def kernel(x, w1, w3, w4):
    global LAST_EXEC_NS, LAST_RESULTS
    nc = _get_compiled()
    xb = np.ascontiguousarray(np.asarray(x, dtype=np.float32)).astype(ml_dtypes.bfloat16)
    wc5, w4b, scal = _prep_weights(w1, w3, w4)

    in_maps = [
        {
            "x_s": np.ascontiguousarray(xb[i * NPC : (i + 1) * NPC]),
            "wc5": wc5,
            "w4b": w4b,
            "scal": scal,
        }
        for i in range(NCORES)
    ]
    if TRACE:
        _enable_trace_hook()
    res = bass_utils.run_bass_kernel_spmd(
        nc,
        in_maps,
        core_ids=list(range(NCORES)),
        trace=TRACE,
        tmpdir=TRACE_DIR,
    )
    LAST_EXEC_NS = res.exec_time_ns
    LAST_RESULTS = res
    out = np.concatenate(
        [res.results[i]["out"].astype(np.float32) for i in range(NCORES)], axis=0
    )
    return out


# revision 31
# speedup vs baseline: 1.0120x; 1.0120x over previous
"""Trainium2 Bass kernel for dense_cnn problem.

Math (per batch element n, C=128 channels, H=W=56, G=8):
  t1 = conv_h(x, w1)          5-tap conv over H with full channel mixing
  t3 = dwconv_h(t1, w3)       3-tap depthwise conv over H
  t4[g] = sum_{c,k} x[c, h, w+2k-2] * w4[c,k,g]   (3 width taps, dil 2)
  out[c] = t3[c] * t4[c % 8]

Device strategy (data-parallel, 4 batch elems per core across 8 cores):
  - PE does ONLY the dense work: t1 as a 5-tap conv (clipped shifted
    matmuls) and t4 broadcast to 128 channels (3 taps) -> 8 column
    passes per chunk instead of the 10 the folded-7-tap version needs.
  - The 3-tap depthwise conv runs on the otherwise-idle vector engines.
    ScalarE makes two per-partition-scaled copies of psA:
      t1s = w3[c,1] * t1      v = w3[c,0] * t1
    then (TensorScalarPtr is not codegen-supported on GpSimd, plain
    tensor_tensor is):
      acc = v[h-1] + t1s[h]                            (GpSimd ADD)
      t3  = (w3[c,2]/w3[c,1]) * t1s[h+1] + acc         (DVE STT)
    t1s/v have zero pad rows, so no border special cases.
    t3 for chunk c needs t1s row h0+8 from chunk c+1's copy, so the
    dw/multiply/store pipeline runs one chunk behind the PE.
  - Final multiply on DVE reads psB (t4) straight from PSUM and writes
    fp16; output DMA'd as fp16 (half the bytes) and widened on host.
  - Matmuls in bf16 (fp32 matmul lowers to a LOW_HIGH pair at <half
    throughput); accumulation stays fp32 in PSUM.
  - Head: input DMAs issued finest-first (x batch 0 in 4 row-slices) so
    the first chunk's data lands ASAP; 7 dummy warm-up matmuls trip the
    PE_HAM clock gate (1.2 -> 2.4 GHz) while the DMAs stream.
  - Tail: the last batch elem stores per-chunk (not per-pair) to cut
    the post-last-matmul drain.
"""

import sys

sys.path.insert(0, "/opt/trn_rl_repo")

import ml_dtypes
import numpy as np

import concourse.bacc as bacc
import concourse.bass as bass
import concourse.mybir as mybir
import concourse.tile as tile
from concourse import bass_utils

N, C, H, W, G = 32, 128, 56, 56, 8
NCORES = 8
NPC = N // NCORES  # batch elems per core
CH = 8             # H rows per chunk
NCHUNK = H // CH

F32 = mybir.dt.float32
F16 = mybir.dt.float16
BF16 = mybir.dt.bfloat16

TRACE = False
TRACE_DIR = None
LAST_EXEC_NS = None
LAST_RESULTS = None

_COMPILED = None


def _enable_trace_hook():
    """The agent image's ``antenv`` lacks ``axon_hooks``, so the boot-time
    NTFF hook registration silently degraded. Recreate the module and
    register the same ctypes-based hook; also skip the bucket upload."""
    import sys as _sys
    import types

    if "antenv.axon_hooks" not in _sys.modules:
        mod = types.ModuleType("antenv.axon_hooks")
        mod._hook = None

        def set_axon_ntff_profile_hook(h):
            mod._hook = h

        def get_axon_ntff_profile_hook():
            return mod._hook

        mod.set_axon_ntff_profile_hook = set_axon_ntff_profile_hook
        mod.get_axon_ntff_profile_hook = get_axon_ntff_profile_hook
        _sys.modules["antenv.axon_hooks"] = mod
        import antenv

        antenv.axon_hooks = mod

    from antenv.axon_hooks import get_axon_ntff_profile_hook as _get

    if _get() is None:
        from trn_agent_boot.trn_boot import _ntff_profile_via_ctypes

        hook = _ntff_profile_via_ctypes("/opt/axon/libaxon_pjrt.so")
        if hook is not None:
            _sys.modules["antenv.axon_hooks"].set_axon_ntff_profile_hook(hook)

    bass_utils.upload_artifacts = lambda tmpdir: f"local:{tmpdir}"


def _t1_matmuls(c, pa, xc, wc_t):
    """5-tap H-conv for chunk c with row clipping at the H borders.
    Output row o of the chunk reads x row 8c+o+e-2 for tap e."""
    h0 = c * CH
    mms = []
    # e=2 covers the full chunk for every c -> emitted first (start=True)
    for e in (2, 0, 1, 3, 4):
        o_lo = max(0, 2 - e - h0)
        o_hi = min(CH, H + 2 - e - h0)
        if o_lo >= o_hi:
            continue
        r0 = h0 + o_lo + e - 2
        r1 = h0 + o_hi + e - 2
        mms.append((wc_t[:, e, :], xc[:, r0:r1, :], pa[:, o_lo:o_hi, :]))
    return mms


def _t4_matmuls(c, pb, xc, w4_t):
    """t4 chunk: 3 width taps at offsets -2/0/+2, col-clipped at borders."""
    h0 = c * CH
    rows = xc[:, h0 : h0 + CH, :]
    return [
        (w4_t[:, 1, :], rows, pb[:]),                               # delta = 0
        (w4_t[:, 0, :], xc[:, h0 : h0 + CH, 0 : W - 2], pb[:, :, 2:W]),   # -2
        (w4_t[:, 2, :], xc[:, h0 : h0 + CH, 2:W], pb[:, :, 0 : W - 2]),   # +2
    ]


def _build():
    nc = bacc.Bacc(
        "TRN2",
        target_bir_lowering=False,
        debug=False,
        enable_asserts=False,
        num_devices=NCORES,
    )

    x_d = nc.dram_tensor("x_s", (NPC, C, H, W), BF16, kind="ExternalInput").ap()
    wc_d = nc.dram_tensor("wc5", (C, 5, C), BF16, kind="ExternalInput").ap()
    w4_d = nc.dram_tensor("w4b", (C, 3, C), BF16, kind="ExternalInput").ap()
    sc_d = nc.dram_tensor("scal", (C, 3), F32, kind="ExternalInput").ap()
    out_d = nc.dram_tensor("out", (NPC, C, H, W), F16, kind="ExternalOutput").ap()

    mult = mybir.AluOpType.mult
    add = mybir.AluOpType.add
    COPY = mybir.ActivationFunctionType.Copy

    with tile.TileContext(nc) as tc:
        with (
            tc.tile_pool(name="wpool", bufs=1) as wpool,
            tc.tile_pool(name="xpool", bufs=1) as xpool,
            tc.tile_pool(name="t1pool", bufs=2) as t1pool,
            tc.tile_pool(name="accpool", bufs=3) as accpool,
            tc.tile_pool(name="t3pool", bufs=3) as t3pool,
            tc.tile_pool(name="opool", bufs=3) as opool,
            tc.tile_pool(name="psA", bufs=3, space="PSUM") as papool,
            tc.tile_pool(name="psB", bufs=5, space="PSUM") as pbpool,
        ):
            # Dummy matmuls while the first DMAs stream in: PE_HAM ungates
            # the 2.4 GHz clock only after ~3us of sustained activity.
            # Results land in a PSUM bank that is never read.
            dmy = wpool.tile([C, 512], BF16)
            nc.vector.memset(dmy[:], 0.0)
            dps = papool.tile([C, CH, W], F32, name="pa")
            for _ in range(7):
                nc.tensor.matmul(
                    dps[:], lhsT=dmy[:, 0:C], rhs=dmy[:, 0 : CH * W],
                    start=True, stop=True,
                )

            wc_t = wpool.tile([C, 5, C], BF16)
            w4_t = wpool.tile([C, 3, C], BF16)
            sc_t = wpool.tile([C, 3], F32)

            xcs = []
            for n in range(NPC):
                xc = xpool.tile([C, H, W], BF16, name=f"xc{n}")
                xcs.append(xc)

            # DMA order: weights first (needed by the very first LDWEIGHTS),
            # then batch 0 in fine row slices so chunk 0 can start ASAP.
            nc.sync.dma_start(wc_t[:], wc_d[:])
            nc.sync.dma_start(xcs[0][:, 0:14, :], x_d[0, :, 0:14, :])
            nc.sync.dma_start(sc_t[:], sc_d[:])
            nc.sync.dma_start(w4_t[:], w4_d[:])
            nc.sync.dma_start(xcs[0][:, 14:28, :], x_d[0, :, 14:28, :])
            nc.sync.dma_start(xcs[0][:, 28:42, :], x_d[0, :, 28:42, :])
            nc.sync.dma_start(xcs[0][:, 42:56, :], x_d[0, :, 42:56, :])
            for n in range(1, NPC):
                nc.sync.dma_start(xcs[n][:, 0:28, :], x_d[n, :, 0:28, :])
                nc.sync.dma_start(xcs[n][:, 28:56, :], x_d[n, :, 28:56, :])

            w31 = sc_t[:, 0:1]
            w30 = sc_t[:, 1:2]
            r2 = sc_t[:, 2:3]

            for n in range(NPC):
                xc = xcs[n]
                last_n = n == NPC - 1

                # t1s rows: 0 = zero pad (h=-1), 1..56 = h, 57 = zero pad
                t1s = t1pool.tile([C, H + 2, W], F32, name="t1s")
                nc.gpsimd.memset(t1s[:, 0:1, :], 0.0)
                nc.gpsimd.memset(t1s[:, H + 1 : H + 2, :], 0.0)
                # v rows: 0 = zero pad (h=-1), 1..56 = h
                v = t1pool.tile([C, H + 1, W], F32, name="v")
                nc.gpsimd.memset(v[:, 0:1, :], 0.0)

                accs = [None] * NCHUNK
                pbs = [None] * NCHUNK
                ots = [None] * NCHUNK

                def emit_front(c):
                    """PE matmuls + Act copy + GpSimd first dw op for chunk c."""
                    h0 = c * CH
                    pa = papool.tile([C, CH, W], F32, name="pa")
                    mms = _t1_matmuls(c, pa, xc, wc_t)
                    for i, (lhsT, rhs, outap) in enumerate(mms):
                        nc.tensor.matmul(
                            outap, lhsT=lhsT, rhs=rhs,
                            start=(i == 0), stop=(i == len(mms) - 1),
                        )
                    pb = pbpool.tile([C, CH, W], F32, name="pb")
                    for i, (lhsT, rhs, outap) in enumerate(_t4_matmuls(c, pb, xc, w4_t)):
                        nc.tensor.matmul(
                            outap, lhsT=lhsT, rhs=rhs,
                            start=(i == 0), stop=(i == 2),
                        )
                    pbs[c] = pb
                    # t1s[1+h0 : 1+h0+CH] = w3_1 * t1   (per-partition scale)
                    nc.scalar.activation(
                        t1s[:, 1 + h0 : 1 + h0 + CH, :], pa[:], COPY, scale=w31
                    )
                    # v[1+h0 : 1+h0+CH] = w3_0 * t1
                    nc.scalar.activation(
                        v[:, 1 + h0 : 1 + h0 + CH, :], pa[:], COPY, scale=w30
                    )
                    # acc = w3_0*t1[h-1] + w3_1*t1[h]
                    acc = accpool.tile([C, CH, W], F32, name="acc")
                    nc.gpsimd.tensor_add(
                        acc[:],
                        v[:, h0 : h0 + CH, :],
                        t1s[:, 1 + h0 : 1 + h0 + CH, :],
                    )
                    accs[c] = acc

                def emit_back(c):
                    """DVE second dw op + final multiply + output DMA for chunk c.
                    Requires chunk c+1's Act copy already emitted (reads t1s
                    row h0+8), except for the last chunk (zero pad row)."""
                    h0 = c * CH
                    t3 = t3pool.tile([C, CH, W], F32, name="t3")
                    nc.vector.scalar_tensor_tensor(
                        t3[:],
                        t1s[:, 2 + h0 : 2 + h0 + CH, :],
                        r2,
                        accs[c][:],
                        op0=mult, op1=add,
                    )
                    ot = opool.tile([C, CH, W], F16, name="ot")
                    nc.vector.tensor_mul(ot[:], t3[:], pbs[c][:])
                    nc.sync.dma_start(out_d[n, :, h0 : h0 + CH, :], ot[:])
                    ots[c] = ot

                for c in range(NCHUNK):
                    emit_front(c)
                    if c >= 1:
                        emit_back(c - 1)
                emit_back(NCHUNK - 1)

    nc.compile()
    return nc


def _get_compiled():
    global _COMPILED
    if _COMPILED is None:
        _COMPILED = _build()
    return _COMPILED


def _prep_weights(w1, w3, w4):
    bf = ml_dtypes.bfloat16
    w1c = np.asarray(w1, dtype=np.float32)[:, :, :, 0]  # (co, ci, 5)
    wc5 = np.ascontiguousarray(np.transpose(w1c, (1, 2, 0))).astype(bf)  # (ci,e,co)
    w4c = np.asarray(w4, dtype=np.float32)[:, :, 0, :]  # (ci, k, g)
    w4b = np.ascontiguousarray(np.tile(w4c, (1, 1, C // G))).astype(bf)
    w3c = np.asarray(w3, dtype=np.float32)[:, 0, :, 0]  # (co, 3)
    w31 = w3c[:, 1].copy()
    w31[np.abs(w31) < 1e-12] = 1e-12
    scal = np.stack([w31, w3c[:, 0], w3c[:, 2] / w31], axis=1)
    return wc5, w4b, np.ascontiguousarray(scal, dtype=np.float32)


def kernel(x, w1, w3, w4):
    global LAST_EXEC_NS, LAST_RESULTS
    nc = _get_compiled()
    xb = np.ascontiguousarray(np.asarray(x, dtype=np.float32)).astype(ml_dtypes.bfloat16)
    wc5, w4b, scal = _prep_weights(w1, w3, w4)

    in_maps = [
        {
            "x_s": np.ascontiguousarray(xb[i * NPC : (i + 1) * NPC]),
            "wc5": wc5,
            "w4b": w4b,
            "scal": scal,
        }
        for i in range(NCORES)
    ]
    if TRACE:
        _enable_trace_hook()
    res = bass_utils.run_bass_kernel_spmd(
        nc,
        in_maps,
        core_ids=list(range(NCORES)),
        trace=TRACE,
        tmpdir=TRACE_DIR,
    )
    LAST_EXEC_NS = res.exec_time_ns
    LAST_RESULTS = res
    out = np.concatenate(
        [res.results[i]["out"].astype(np.float32) for i in range(NCORES)], axis=0
    )
    return out
